# revision 1
# baseline (speedup 1.0000x reference)
"""Distributed Trainium2 kernel: Gemma-style attention block (B=2,T=2048,H=2048,
NH=16,NKV=4,HD=128), tensor-parallel over heads across 8 NeuronCores.

Per core c: q heads {2c, 2c+1}, kv head c//2.  Activations are kept
feature-major ("transposed", [d_part, t_free]) so every matmul contracts on the
partition dim.  Softmax is max-free (safe: rmsnorm bounds |scores| <= sqrt(HD)),
denominators and rmsnorm sum-of-squares are computed pre-broadcast via an
all-ones 128x128 stationary matmul.  o_proj partials are summed on host.
"""

import os
import sys

sys.path.insert(0, "/opt/trn_rl_repo")

import numpy as np
import ml_dtypes

import concourse.bass as bass
import concourse.mybir as mybir
import concourse.tile as tile
from concourse.bass_utils import run_bass_kernel_spmd

BF16 = ml_dtypes.bfloat16

B, T, H = 2, 2048, 2048
NH, NKV, HD = 16, 4, 128
THETA = 10000.0
EPS = 1e-6
NCORES = 8
QH = NH // NCORES          # 2 q heads per core
BT = B * T                 # 4096
NBLK = T // 512            # 4 blocks of 512 per batch
SCALE = 1.0 / np.sqrt(HD)

LAST_RESULTS = None        # stash for test harness profiling

# column offsets inside the packed constants tile [128, NCONST]
OFF_WQKV = 0                      # 16*512: wqkvT tiles, [p, ht*512+j] = WqkvT[ht*128+p, j]
OFF_WO = OFF_WQKV + 16 * 512      # QH*2048
OFF_CQ = OFF_WO + QH * H          # 2048
OFF_CK = OFF_CQ + T               # 2048
OFF_SIN = OFF_CK + T              # 2048
OFF_RQ = OFF_SIN + T              # 128
OFF_RK = OFF_RQ + HD              # 128
OFF_MASK = OFF_RK + HD            # 4*512
OFF_ONES = OFF_MASK + 4 * 512     # 128
NCONST = OFF_ONES + 128


def _rope_tables(w_q, w_k):
    """rope(w*q) = cosw ⊙ q + sin ⊙ (R_w @ q) where cosw = cos·(1+w) and
    R_w = rot_half matrix with the ±1 and the (1+w) source weight folded in.
    Returns cosw_q, cosw_k, sin (plain), rotmT_q, rotmT_k (lhsT layout)."""
    inv = 1.0 / (THETA ** (np.arange(0, HD, 2, dtype=np.float64) / HD))  # [64]
    t = np.arange(T, dtype=np.float64)
    fr = np.outer(inv, t)                      # [64, T]
    emb = np.concatenate([fr, fr], 0)          # [HD, T]
    cos, sin = np.cos(emb), np.sin(emb)
    cosws, rotms = [], []
    for w in (w_q, w_k):
        wp = 1.0 + w.astype(np.float64)
        cosws.append((cos * wp[:, None]).astype(BF16))
        R = np.zeros((HD, HD))
        for m in range(64):
            R[m, m + 64] = -wp[m + 64]
        for m in range(64, HD):
            R[m, m - 64] = +wp[m - 64]
        rotms.append(np.ascontiguousarray(R.T).astype(BF16))  # lhsT[k, m] = R[m, k]
    return cosws[0], cosws[1], sin.astype(BF16), rotms[0], rotms[1]


def _legalize_waits(nc):
    """This container's walrus accepts only ONE sync wait per instruction
    (even shipped Tile kernels fail codegen). Split each multi-wait
    instruction into single-wait NOPs on the same engine followed by the
    original holding the last wait — per-engine program order makes this
    exactly equivalent."""
    nid = 0
    for fn in nc.m.functions:
        for blk in fn.blocks:
            out = []
            for inst in blk.instructions:
                si = getattr(inst, "sync_info", None)
                if si is not None and si.on_wait and len(si.on_wait) > 1:
                    waits = list(si.on_wait)
                    ups = list(si.on_update) if si.on_update else []
                    for w in waits[:-1]:
                        nop = mybir.InstNoOp(name=f"swx-{nid}", ins=[], outs=[])
                        nid += 1
                        nop.engine = inst.engine
                        nop.sync_info = mybir.SyncInfo(on_wait=[w], on_update=[])
                        out.append(nop)
                    inst.sync_info = mybir.SyncInfo(
                        on_wait=[waits[-1]], on_update=ups)
                out.append(inst)
            blk.instructions = out
    return nc


def _build_graph(perturb=0, repeat=1, cfg=None):
    cfg = {**dict(xtp=32, tmp=6, pacc=2, pmm=4, depth=3, fuse3=0), **(cfg or {})}
    nc = bass.Bass()
    f32, bf16 = mybir.dt.float32, mybir.dt.bfloat16

    xT = nc.dram_tensor("xT", [H, BT], bf16, kind="ExternalInput")
    consts = nc.dram_tensor("consts", [128, NCONST], bf16, kind="ExternalInput")
    out = nc.dram_tensor("out", [BT, H], bf16, kind="ExternalOutput")

    with tile.TileContext(nc) as tc:
        with (
            tc.tile_pool(name="singles", bufs=1) as singles,
            tc.tile_pool(name="xtp", bufs=cfg["xtp"]) as xtp,
            tc.tile_pool(name="tmp", bufs=cfg["tmp"]) as tmp,
            tc.tile_pool(name="psum", bufs=cfg["pacc"], space="PSUM") as pacc,
            tc.tile_pool(name="psmm", bufs=cfg["pmm"], space="PSUM") as pmm,
        ):
            # ---- resident constants: ONE dma -> one queue semaphore ----
            consts_sb = singles.tile([128, NCONST], bf16)
            nc.sync.dma_start(out=consts_sb, in_=consts[:, :])
            wqkv_sb = consts_sb[:, OFF_WQKV:OFF_WQKV + 16 * 512]
            wo_sb = consts_sb[:, OFF_WO:OFF_WO + QH * H]
            cq_sb = consts_sb[:, OFF_CQ:OFF_CQ + T]
            ck_sb = consts_sb[:, OFF_CK:OFF_CK + T]
            sin_sb = consts_sb[:, OFF_SIN:OFF_SIN + T]
            rq_sb = consts_sb[:, OFF_RQ:OFF_RQ + HD]
            rk_sb = consts_sb[:, OFF_RK:OFF_RK + HD]
            mask_sb = consts_sb[:, OFF_MASK:OFF_MASK + 4 * 512]
            ones_sb = consts_sb[:, OFF_ONES:OFF_ONES + 128]
            for _ in range(perturb):
                nc.sync.nop()

            # ---- per-batch activations (feature-major) ----
            qT = [singles.tile([128, QH * T], bf16, name=f"qT{b}", tag=f"qT{b}")
                  for b in range(B)]
            kT = [singles.tile([128, T], bf16, name=f"kT{b}", tag=f"kT{b}")
                  for b in range(B)]
            vn = [singles.tile([128, 16 * 128], bf16, name=f"vn{b}", tag=f"vn{b}")
                  for b in range(B)]
            attnT = [singles.tile([128, QH * T], bf16, name=f"attnT{b}", tag=f"attnT{b}")
                     for b in range(B)]

            def phase1(b):
                for blk in range(NBLK):
                    t0 = blk * 512
                    bt0 = b * T + t0
                    xts = []
                    for ht in range(16):
                        xt_t = xtp.tile([128, 512], bf16, tag="xt")
                        nc.sync.dma_start(
                            out=xt_t, in_=xT[ht * 128:(ht + 1) * 128, bt0:bt0 + 512])
                        xts.append(xt_t)
                    # q0, q1, k projections (feature-major out)
                    for dt in range(3):
                        ps = pacc.tile([128, 512], f32, tag="acc")
                        for ht in range(16):
                            nc.tensor.matmul(
                                ps,
                                lhsT=wqkv_sb[:, ht * 512 + dt * 128:ht * 512 + (dt + 1) * 128],
                                rhs=xts[ht], start=(ht == 0), stop=(ht == 15))
                        traw = tmp.tile([128, 512], bf16, tag="traw")
                        with nc.allow_low_precision(reason="bf16 act copy"):
                            nc.vector.tensor_copy(out=traw, in_=ps)
                        sq = tmp.tile([128, 512], bf16, tag="sq")
                        nc.vector.tensor_mul(sq, traw, traw)
                        ssq = pmm.tile([128, 512], f32, tag="mm")
                        nc.tensor.matmul(ssq, lhsT=ones_sb, rhs=sq, start=True, stop=True)
                        std = tmp.tile([128, 512], f32, tag="std")
                        nc.scalar.activation(
                            out=std, in_=ssq,
                            func=mybir.ActivationFunctionType.Sqrt,
                            scale=1.0 / HD)
                        rstd = tmp.tile([128, 512], bf16, tag="rstd")
                        with nc.allow_low_precision(reason="rstd bf16 ok at 2e-2 tol"):
                            nc.vector.reciprocal(out=rstd, in_=std)
                        cos_t, rot_t = (cq_sb, rq_sb) if dt < 2 else (ck_sb, rk_sb)
                        t1 = tmp.tile([128, 512], bf16, tag="t1")
                        nc.vector.tensor_mul(t1, traw, cos_t[:, t0:t0 + 512])
                        rps = pmm.tile([128, 512], f32, tag="mm")
                        nc.tensor.matmul(rps, lhsT=rot_t, rhs=traw, start=True, stop=True)
                        t2 = tmp.tile([128, 512], bf16, tag="t2")
                        nc.vector.tensor_mul(t2, rps, sin_sb[:, t0:t0 + 512])
                        nc.vector.tensor_add(out=t1, in0=t1, in1=t2)
                        dest = (qT[b][:, dt * T + t0:dt * T + t0 + 512] if dt < 2
                                else kT[b][:, t0:t0 + 512])
                        nc.vector.tensor_mul(dest, t1, rstd)
                    # v projection, natural layout [t_part, d_free]
                    vps = pacc.tile([128, 512], f32, tag="acc")
                    for c4 in range(4):
                        for ht in range(16):
                            nc.tensor.matmul(
                                vps[:, c4 * 128:(c4 + 1) * 128],
                                lhsT=xts[ht][:, c4 * 128:(c4 + 1) * 128],
                                rhs=wqkv_sb[:, ht * 512 + 384:ht * 512 + 512],
                                start=(ht == 0), stop=(ht == 15))
                    with nc.allow_low_precision(reason="bf16 act copy"):
                        nc.vector.tensor_copy(
                            out=vn[b][:, blk * 512:(blk + 1) * 512], in_=vps)

            def attn_block(b, h, j):
                # Software-pipelined: S^T matmuls issued DEPTH tiles ahead so
                # the PE never stalls on the ACT exp of the current tile.
                DEPTH = cfg["depth"]
                if True:
                    if True:
                        ntk = 4 * j + 4
                        aps = pacc.tile([128, 512], f32, tag="acc")
                        dps = pacc.tile([128, 512], f32, tag="den")
                        sps_l, pt_l = [], []

                        def issue_st(i):
                            sps = pmm.tile([128, 512], f32, tag="mm", name="sps")
                            nc.tensor.matmul(
                                sps, lhsT=kT[b][:, i * 128:(i + 1) * 128],
                                rhs=qT[b][:, h * T + j * 512:h * T + (j + 1) * 512],
                                start=True, stop=True)
                            sps_l.append(sps)

                        def issue_exp(i):
                            pt = tmp.tile([128, 512], bf16, tag="pt", name="pt")
                            nc.scalar.activation(
                                out=pt, in_=sps_l[i],
                                func=mybir.ActivationFunctionType.Exp, scale=SCALE)
                            if i >= 4 * j:
                                r = i - 4 * j
                                nc.vector.tensor_mul(
                                    pt, pt, mask_sb[:, r * 512:(r + 1) * 512])
                            pt_l.append(pt)

                        for i in range(min(DEPTH, ntk)):
                            issue_st(i)
                        issue_exp(0)
                        for i in range(ntk):
                            if i + DEPTH < ntk:
                                issue_st(i + DEPTH)
                            if i + 1 < ntk:
                                issue_exp(i + 1)
                            nc.tensor.matmul(dps, lhsT=ones_sb, rhs=pt_l[i],
                                             start=(i == 0), stop=(i == ntk - 1))
                            nc.tensor.matmul(aps, lhsT=vn[b][:, i * 128:(i + 1) * 128],
                                             rhs=pt_l[i], start=(i == 0),
                                             stop=(i == ntk - 1))
                        recip = tmp.tile([128, 512], mybir.dt.float32, tag="rec")
                        nc.vector.reciprocal(out=recip, in_=dps)
                        nc.vector.tensor_mul(
                            attnT[b][:, h * T + j * 512:h * T + (j + 1) * 512], aps, recip)

            def phase2(b):
                for h in range(QH):
                    for j in range(NBLK):
                        attn_block(b, h, j)

            def oproj_tile(b, m, j):
                ops = pmm.tile([128, 512], f32, tag="mm", name="ops")
                for hh in range(QH):
                    nc.tensor.matmul(
                        ops,
                        lhsT=attnT[b][:, hh * T + m * 128:hh * T + (m + 1) * 128],
                        rhs=wo_sb[:, hh * H + j * 512:hh * H + (j + 1) * 512],
                        start=(hh == 0), stop=(hh == QH - 1))
                osb = tmp.tile([128, 512], bf16, tag="osb", name="osb")
                with nc.allow_low_precision(reason="bf16 partials, host-summed f32"):
                    nc.vector.tensor_copy(out=osb, in_=ops)
                nc.sync.dma_start(
                    out=out[b * T + m * 128:b * T + (m + 1) * 128,
                            j * 512:(j + 1) * 512],
                    in_=osb)

            def phase3(b):
                for m in range(16):
                    for j in range(NBLK):
                        oproj_tile(b, m, j)

            def phase23_fused(b):
                for j in range(NBLK):
                    for h in range(QH):
                        attn_block(b, h, j)
                    for m in range(4 * j, 4 * j + 4):
                        for jo in range(NBLK):
                            oproj_tile(b, m, jo)

            for _ in range(repeat):   # >1 only for benchmarking (idempotent)
                if cfg["fuse3"]:
                    phase1(0)
                    phase23_fused(0)
                    phase1(1)
                    phase23_fused(1)
                else:
                    phase1(0)
                    phase2(0)
                    phase1(1)
                    phase3(0)
                    phase2(1)
                    phase3(1)
    return nc


_GRAPH = None


def kernel(x, Wq, Wk, Wv, Wo, q_norm_w, k_norm_w):
    global _GRAPH, LAST_RESULTS
    x = np.asarray(x, dtype=np.float32)
    Wq = np.asarray(Wq, dtype=np.float32)
    Wk = np.asarray(Wk, dtype=np.float32)
    Wv = np.asarray(Wv, dtype=np.float32)
    Wo = np.asarray(Wo, dtype=np.float32)
    q_norm_w = np.asarray(q_norm_w, dtype=np.float32)
    k_norm_w = np.asarray(k_norm_w, dtype=np.float32)

    xT = np.ascontiguousarray(x.reshape(BT, H).T).astype(BF16)
    cos_q, cos_k, sin_d, rotm_q, rotm_k = _rope_tables(q_norm_w, k_norm_w)
    p = np.arange(128)[:, None]
    f = np.arange(512)[None, :]
    masks = np.stack([(f >= 128 * r + p) for r in range(4)]).astype(BF16)
    masks_cols = masks.transpose(1, 0, 2).reshape(128, 4 * 512)

    in_maps = []
    for c in range(NCORES):
        kv = c // 2
        w_all = np.concatenate([
            Wq[QH * HD * c:QH * HD * (c + 1)],
            Wk[HD * kv:HD * (kv + 1)],
            Wv[HD * kv:HD * (kv + 1)]], 0)              # [512, H]
        wqkvT = np.ascontiguousarray(w_all.T).astype(BF16)       # [H, 512]
        woT = np.ascontiguousarray(
            Wo[:, QH * HD * c:QH * HD * (c + 1)].T).astype(BF16)  # [QH*HD, H]
        consts = np.zeros((128, NCONST), dtype=BF16)
        consts[:, OFF_WQKV:OFF_WQKV + 16 * 512] = (
            wqkvT.reshape(16, 128, 512).transpose(1, 0, 2).reshape(128, 16 * 512))
        consts[:, OFF_WO:OFF_WO + QH * H] = (
            woT.reshape(QH, 128, H).transpose(1, 0, 2).reshape(128, QH * H))
        consts[:, OFF_CQ:OFF_CQ + T] = cos_q
        consts[:, OFF_CK:OFF_CK + T] = cos_k
        consts[:, OFF_SIN:OFF_SIN + T] = sin_d
        consts[:, OFF_RQ:OFF_RQ + HD] = rotm_q
        consts[:, OFF_RK:OFF_RK + HD] = rotm_k
        consts[:, OFF_MASK:OFF_MASK + 4 * 512] = masks_cols
        consts[:, OFF_ONES:OFF_ONES + 128] = 1.0
        in_maps.append({"xT": xT, "consts": consts})

    if _GRAPH is None:
        _GRAPH = _legalize_waits(_build_graph())

    want_trace = bool(int(os.environ.get("ATTN_TRACE", "0")))
    try:
        res = run_bass_kernel_spmd(
            _GRAPH, in_maps, core_ids=list(range(NCORES)), trace=want_trace)
    except ModuleNotFoundError:
        if not want_trace:
            raise
        # axon NTFF profile hook unavailable in this environment
        res = run_bass_kernel_spmd(
            _GRAPH, in_maps, core_ids=list(range(NCORES)), trace=False)
    LAST_RESULTS = res
    acc = np.zeros((BT, H), dtype=np.float32)
    for r in res.results:
        acc += r["out"]
    return acc.reshape(B, T, H)

